# revision 18
# baseline (speedup 1.0000x reference)
"""Multi-head self-attention (B=1, S=4096, D=2048, H=16, rotary_dim=64) on 8 TRN2 NeuronCores.

Head-sharded tensor parallelism: each core computes 2 heads end-to-end
(QKV projection + RoPE + full softmax attention) plus its slice of the
row-sharded output projection; the 8 partial [S, D] outputs are summed on
the host.

Precision/perf scheme:
  - All matmuls in fp16 (PE fp16 = full 1 elem/cell/cycle rate; fp8 was
    measured to push its ~3% element error through the softmax average at
    full relative strength -> 6% output error, over the 2e-2 gate).
  - Softmax skips max-subtraction (scores ~N(0,1)): exp via ScalarE in
    2048-wide activations over 4-bank PSUM score quads (amortizes the
    ~293ns per-ACTIVATE overhead). Denominator = serial chain of 7 DVE
    quad-adds + 4 ones-matmuls per (head, q-tile), replacing the old
    DVE/GpSimd pair-add scheme (frees ~220us GpSimd + ~90us DVE).
  - Out-proj partials evicted PSUM->SBUF alternating DVE/ScalarE, written
    to DRAM as fp16; host sums the 8 partials in f32.
"""

import numpy as np

import concourse.bass as bass
import concourse.mybir as mybir
import concourse.tile as tile
from concourse import bacc
from concourse.bass_utils import run_bass_kernel_spmd
F32 = mybir.dt.float32
FP16 = mybir.dt.float16
FP8 = mybir.dt.float8e4

D = 2048
H = 16
HD = 128
ROT = 64
NCORES = 8
HPC = H // NCORES  # heads per core
SCALE = float(HD) ** -0.5
WS = 1.0  # no operand pre-scale in the all-fp16 scheme

DR = mybir.MatmulPerfMode.DoubleRow

_CACHE = {}


def build_module(S=4096, ST=512, QTL=512):
    """Build the per-core SPMD bass module. Returns compiled nc.

    3-stage schedule to keep the PE busy across phases:
      A: QKV+RoPE for head 0
      B: attention(head0, qt) interleaved with QKV+RoPE(head1) s-tiles
      C: attention(head1, qt) interleaved with the output projection
    """
    NST = S // ST        # QKV s-tiles
    NKT = D // 128       # contraction tiles for QKV
    NQT = S // QTL       # attention q-tiles
    NKC = S // 128       # attention k-chunks
    NQUAD = NKC // 4     # 4-chunk score quads per q-tile
    ETL = 512            # out-proj e-tile
    NET = D // ETL
    assert NST == NQT

    nc = bacc.Bacc(None, target_bir_lowering=False, debug=True)

    xT_d = nc.dram_tensor("xT", [D, S], FP16, kind="ExternalInput")
    w_d = nc.dram_tensor("wsl", [D, 3 * HPC, 128], FP16, kind="ExternalInput")
    wo_d = nc.dram_tensor("wout", [HPC * HD, D], FP16, kind="ExternalInput")
    b_d = nc.dram_tensor("bsl", [128, 3 * HPC], F32, kind="ExternalInput")
    cs_d = nc.dram_tensor("cs", [ROT, 2, S], FP16, kind="ExternalInput")
    y_d = nc.dram_tensor("y", [S, D], FP16, kind="ExternalOutput")

    xT_r = xT_d[:].rearrange("(t p) s -> p t s", p=128)             # [128,16,S]
    w_r = w_d[:].rearrange("(t p) h m -> p t h m", p=128)            # [128,16,6,128]
    wo_r = wo_d[:].rearrange("(h p) e -> p h e", p=128)              # [128,HPC,D]

    with tile.TileContext(nc) as tc:
        with (
            tc.tile_pool(name="persist", bufs=1) as P,
            tc.tile_pool(name="csp", bufs=1) as csp,
            tc.tile_pool(name="xp", bufs=2) as xp,
            tc.tile_pool(name="vtp", bufs=2) as vtp,
            tc.tile_pool(name="rtp", bufs=2) as rtp,
            tc.tile_pool(name="ptp", bufs=4) as ptp,
            tc.tile_pool(name="accp", bufs=2) as accp,
            tc.tile_pool(name="rcp", bufs=2) as rcp,
            tc.tile_pool(name="yp", bufs=3) as yp,
            # PSUM: psA(2 shared slots: QKV groups / V-transpose / out-proj)
            # + score pairs (2x2 banks) + oacc (1) + dn (1) = 8 banks
            tc.tile_pool(name="psA", bufs=2, space="PSUM") as psA,
            tc.tile_pool(name="pss", bufs=2, space="PSUM") as pssp,
            tc.tile_pool(name="pso", bufs=1, space="PSUM") as psop,
            tc.tile_pool(name="psd", bufs=1, space="PSUM") as psdp,
        ):
            QT = [P.tile([128, S], FP16, tag=f"qt{h}", name=f"qt{h}") for h in range(HPC)]
            KT = [P.tile([128, S], FP16, tag=f"kt{h}", name=f"kt{h}") for h in range(HPC)]
            V = [P.tile([128, NKC, 128], FP16, tag=f"v{h}", name=f"v{h}") for h in range(HPC)]
            AT = [P.tile([128, S], FP16, tag=f"at{h}", name=f"at{h}") for h in range(HPC)]
            w_sb = P.tile([128, NKT, 3 * HPC, 128], FP16, tag="wsb", name="w_sb")
            wo_sb = P.tile([128, HPC, D], FP16, tag="wosb", name="wo_sb")
            b_sb = P.tile([128, 3 * HPC], F32)
            onesw = P.tile([128, 128], FP16)
            cs_sb = csp.tile([ROT, 2, S], FP16, tag="cst", name="cs_sb")

            # Initial loads in consumption order on the two hardware DMA
            # queues (sync/scalar): the first QKV matmul needs w[j=0] and the
            # first quarter of x-tile (0,0); later weights follow behind.
            nkq = NKT // 2
            def w_load(jj):
                for i in range(2):
                    eng = nc.sync if i % 2 == 0 else nc.scalar
                    eng.dma_start(
                        w_sb[:, i * nkq : (i + 1) * nkq, jj : jj + 1, :],
                        w_r[:, i * nkq : (i + 1) * nkq, jj : jj + 1, :])
            w_load(0)
            xt00 = xp.tile([128, NKT, ST], FP16, tag="xt", name="xt_0_0")
            for i in range(4):
                eng = nc.sync if i % 2 == 0 else nc.scalar
                q = NKT // 4
                eng.dma_start(xt00[:, i * q : (i + 1) * q, :],
                              xT_r[:, i * q : (i + 1) * q, bass.ts(0, ST)])
            w_load(1)
            w_load(2)
            nc.sync.dma_start(b_sb[:], b_d[:])
            nc.scalar.dma_start(cs_sb[:, :, 0 : S // 2], cs_d[:, :, 0 : S // 2])
            nc.sync.dma_start(cs_sb[:, :, S // 2 :], cs_d[:, :, S // 2 :])
            for jj in range(3, 6):
                w_load(jj)
            nc.scalar.dma_start(wo_sb[:], wo_r)
            nc.vector.memset(onesw[:], WS)

            # Warm the PE clock (HAM) during the initial DMAs: ~3.5us of
            # dummy matmuls so the first real matmuls run at 2.4 GHz.
            wm = psdp.tile([128, 512], F32, tag="dn", name="warm")
            for i in range(40):
                nc.tensor.matmul(wm[:, 0:128], onesw[:], onesw[:],
                                 start=(i == 0), stop=(i == 39))
            wmr = rcp.tile([128, 1], F32, tag="rc", name="warmread")
            nc.vector.tensor_copy(wmr[:], wm[:, 0:1])

            def qkv_stile_thunks(h, st):
                """QKV projection (fp8 DoubleRow) + RoPE + V transpose for one
                head / s-tile, returned as emission thunks for interleaving."""
                sl = bass.ts(st, ST)
                j = 3 * h
                state = {}
                thunks = []

                def t_load():
                    if h == 0 and st == 0:
                        state["xt"] = xt00  # preloaded in the preamble
                        return
                    xt = xp.tile([128, NKT, ST], FP16, tag="xt", name=f"xt_{h}_{st}")
                    nc.sync.dma_start(xt[:, 0 : NKT // 2, :], xT_r[:, 0 : NKT // 2, sl])
                    nc.scalar.dma_start(xt[:, NKT // 2 :, :], xT_r[:, NKT // 2 :, sl])
                    state["xt"] = xt
                thunks.append(t_load)

                def t_group_open(which):
                    state[f"ps{which}"] = psA.tile(
                        [128, ST], F32, tag="a", name=f"ps{which}_{h}_{st}")
                def t_mms(which, k0, k1):
                    ps = state[f"ps{which}"]
                    xt = state["xt"]
                    for k in range(k0, k1):
                        nc.tensor.matmul(
                            ps[:], w_sb[:, k, j + which, :], xt[:, k, :],
                            start=(k == 0), stop=(k == NKT - 1),
                        )
                def t_evict_qk(which):
                    dst = QT[h] if which == 0 else KT[h]
                    nc.scalar.activation(
                        dst[:, sl], state[f"ps{which}"][:],
                        mybir.ActivationFunctionType.Identity,
                        bias=b_sb[:, j + which : j + which + 1],
                    )
                    tmp = rtp.tile([ROT, ST], FP16, tag="rtmp", name=f"rt_{h}_{st}_{which}")
                    nc.vector.tensor_copy(tmp[0 : ROT // 2, :], dst[ROT // 2 : ROT, sl])
                    nc.vector.tensor_copy(tmp[ROT // 2 : ROT, :], dst[0 : ROT // 2, sl])
                    nc.vector.tensor_mul(tmp[:], tmp[:], cs_sb[:, 1, sl])
                    nc.vector.tensor_mul(dst[0:ROT, sl], dst[0:ROT, sl], cs_sb[:, 0, sl])
                    nc.vector.tensor_add(dst[0:ROT, sl], dst[0:ROT, sl], tmp[:])
                def t_evict_v():
                    vt = vtp.tile([128, ST], FP16, tag="vt", name=f"vt_{h}_{st}")
                    nc.scalar.activation(
                        vt[:], state["ps2"][:], mybir.ActivationFunctionType.Identity,
                        bias=b_sb[:, j + 2 : j + 3],
                    )
                    state["vt"] = vt
                def t_vtr():
                    # V transpose via the DMA crossbar (no PE/ACT involvement)
                    for sc in range(ST // 128):
                        eng = nc.sync if sc % 2 == 0 else nc.scalar
                        eng.dma_start_transpose(
                            V[h][:, st * (ST // 128) + sc, :],
                            state["vt"][:, bass.ts(sc, 128)])

                for which in range(3):
                    thunks.append(lambda w=which: t_group_open(w))
                    for k0 in range(0, NKT, 4):
                        thunks.append(lambda w=which, a=k0: t_mms(w, a, a + 4))
                    if which < 2:
                        thunks.append(lambda w=which: t_evict_qk(w))
                thunks.append(t_evict_v)
                thunks.append(t_vtr)
                return thunks

            def qkv_stile(h, st):
                for t in qkv_stile_thunks(h, st):
                    t()

            NPAIR = NKC // 2

            def attn_iter(h, qt, fillers=()):
                """One attention iteration: 512 queries x full S keys, in
                2-chunk score pairs (2 PSUM banks each, double-buffered so
                QK of pair i+1 overlaps the exp of pair i).
                `fillers` are extra emission thunks interleaved between pairs."""
                fillers = list(fillers)
                fi = 0
                qsl = bass.ts(qt, QTL)
                oacc = psop.tile([128, QTL], F32, tag="oacc", name=f"oacc_{h}_{qt}")
                acc = accp.tile([128, 2, QTL], FP16, tag="acc", name=f"acc_{h}_{qt}")
                pts = {}
                for pair in range(NPAIR + 1):
                    if pair < NPAIR:
                        # scores for 2 k-chunks into a 2-bank PSUM pair
                        pss = pssp.tile([128, 2, QTL], F32, tag="pss",
                                        name=f"pss_{qt}_{h}_{pair}")
                        for c in range(2):
                            nc.tensor.matmul(
                                pss[:, c, :],
                                KT[h][:, bass.ts(2 * pair + c, 128)], QT[h][:, qsl],
                                start=True, stop=True,
                            )
                        pt = ptp.tile([128, 2, QTL], FP16, tag="pt",
                                      name=f"pt_{qt}_{h}_{pair}")
                        pts[pair] = pt
                        nc.scalar.activation(
                            pt[:], pss[:],
                            mybir.ActivationFunctionType.Exp, scale=SCALE / (WS * WS),
                        )
                    while fi < len(fillers) and fi * (NPAIR + 1) <= (pair + 1) * len(fillers):
                        fillers[fi]()
                        fi += 1
                    qd = pair - 1
                    if 0 <= qd < NPAIR:
                        pt = pts.pop(qd)
                        for c in range(2):
                            kd = 2 * qd + c
                            nc.tensor.matmul(
                                oacc[:], V[h][:, kd, :], pt[:, c, :],
                                start=(kd == 0), stop=(kd == NKC - 1),
                            )
                        # denominator accumulation (serial chain on DVE)
                        if qd == 0:
                            pts["acc_prev"] = pt  # defer: first add needs pair 1
                        elif qd == 1:
                            nc.vector.tensor_add(acc[:], pts.pop("acc_prev")[:], pt[:])
                        else:
                            nc.vector.tensor_add(acc[:], acc[:], pt[:])
                while fi < len(fillers):
                    fillers[fi]()
                    fi += 1

                # tail: denominator matmuls + reciprocal + normalize. Returned
                # as thunks so the caller can weave them into the NEXT
                # iteration's filler stream (hides the serial dn/recip chain
                # behind the next iteration's QK/exp ramp-up).
                def t_dn():
                    dn = psdp.tile([128, QTL], F32, tag="dn", name=f"dn_{h}_{qt}")
                    for c in range(2):
                        nc.tensor.matmul(dn[:], onesw[:], acc[:, c, :],
                                         start=(c == 0), stop=(c == 1))
                    state["dn"] = dn
                def t_recip():
                    rc = rcp.tile([128, QTL], F32, tag="rc", name=f"rc_{h}_{qt}")
                    scr = rcp.tile([128, QTL], F32, tag="rcscr", name=f"rs_{h}_{qt}")
                    nc.vector.reciprocal_approx_accurate(rc[:], state["dn"][:], scr[:])
                    state["rc"] = rc
                def t_mul(c4):
                    msl = bass.ds(qt * QTL + c4 * 128, 128)
                    nc.vector.tensor_mul(AT[h][:, msl], oacc[:, bass.ts(c4, 128)],
                                         state["rc"][:, bass.ts(c4, 128)])
                state = {}
                return [t_dn, t_recip] + [lambda c=c: t_mul(c) for c in range(4)]

            def outproj_thunks(qt):
                thunks = []
                for sc4 in range(QTL // 128):
                    for et in range(NET):
                        def blk(sc4=sc4, et=et):
                            ssl = bass.ds(qt * QTL + sc4 * 128, 128)
                            esl = bass.ts(et, ETL)
                            psy = psA.tile([128, ETL], F32, tag="a",
                                           name=f"psy_{qt}_{sc4}_{et}")
                            for h in range(HPC):
                                nc.tensor.matmul(
                                    psy[:], AT[h][:, ssl], wo_sb[:, h, esl],
                                    start=(h == 0), stop=(h == HPC - 1),
                                )
                            yt = yp.tile([128, ETL], FP16, tag="yt",
                                         name=f"yt_{qt}_{sc4}_{et}")
                            k = sc4 * NET + et
                            if k % 2 == 0:
                                nc.vector.tensor_copy(yt[:], psy[:])
                            else:
                                nc.scalar.activation(
                                    yt[:], psy[:], mybir.ActivationFunctionType.Copy)
                            (nc.sync if k % 2 == 0 else nc.scalar).dma_start(
                                y_d[ssl, esl], yt[:])
                        thunks.append(blk)
                return thunks

            # stage A: QKV head 0
            for st in range(NST):
                qkv_stile(0, st)
            # stage B: attention(head0) || QKV head 1; each iteration's tail
            # (dn/recip/normalize) is woven into the next iteration's fillers
            tail = []
            for qt in range(NQT):
                tail = attn_iter(0, qt, fillers=tail + qkv_stile_thunks(1, qt))
            # stage C: attention(head1) || output projection (lagged one
            # q-tile so fillers never wait on the current iteration's AT)
            for qt in range(NQT):
                fill = tail + (outproj_thunks(qt - 1) if qt > 0 else [])
                tail = attn_iter(1, qt, fillers=fill)
            for t in tail + outproj_thunks(NQT - 1):
                t()

    nc.compile()
    return nc


def _host_prep(x, w_qkv, b_qkv, w_out, S):
    """Build per-core input maps."""
    xT = np.ascontiguousarray(x.reshape(S, D).T).astype(np.float16)

    # RoPE tables (match reference._rope_cos_sin)
    inv_freq = (1.0 / (10000.0 ** (np.arange(0, ROT, 2, dtype=np.float32) / ROT))).astype(np.float32)
    t = np.arange(S, dtype=np.float32)
    freqs = np.outer(t, inv_freq)                      # [S, ROT/2]
    emb = np.concatenate([freqs, freqs], axis=-1)      # [S, ROT]
    cosT = np.cos(emb).astype(np.float32).T            # [ROT, S]
    sinT = np.sin(emb).astype(np.float32).T
    sinS = sinT.copy()
    sinS[0 : ROT // 2] *= -1.0
    cs = np.ascontiguousarray(np.stack([cosT, sinS], axis=1)).astype(np.float16)  # [ROT, 2, S]

    in_maps = []
    for c in range(NCORES):
        cols = []
        bcols = []
        for h in [HPC * c + i for i in range(HPC)]:
            for part in range(3):  # q, k, v
                off = part * D + h * HD
                cols.append(w_qkv[:, off : off + HD] * WS)
                bcols.append(b_qkv[off : off + HD] * WS)
        wsl = np.ascontiguousarray(np.stack(cols, axis=1)).astype(np.float16)   # [D, 3*HPC, 128]
        bsl = np.ascontiguousarray(np.stack(bcols, axis=1)).astype(np.float32)   # [128, 3*HPC]
        wout_sl = np.ascontiguousarray(w_out[c * HPC * HD : (c + 1) * HPC * HD, :]).astype(np.float16)
        in_maps.append({"xT": xT, "wsl": wsl, "bsl": bsl, "wout": wout_sl, "cs": cs})
    return in_maps


def kernel(x, w_qkv, b_qkv, w_out, b_out):
    B, S, D_ = x.shape
    assert B == 1 and D_ == D
    if "nc" not in _CACHE:
        _CACHE["nc"] = build_module(S=S)
    nc = _CACHE["nc"]
    in_maps = _host_prep(np.asarray(x, dtype=np.float32), np.asarray(w_qkv, dtype=np.float32),
                         np.asarray(b_qkv, dtype=np.float32), np.asarray(w_out, dtype=np.float32), S)
    res = run_bass_kernel_spmd(nc, in_maps, list(range(NCORES)))
    y = np.zeros((S, D), dtype=np.float32)
    for c in range(NCORES):
        y += res.results[c]["y"].astype(np.float32)
    y += np.asarray(b_out, dtype=np.float32)[None, :]
    return y.reshape(1, S, D)


# revision 19
# speedup vs baseline: 1.0352x; 1.0352x over previous
"""Multi-head self-attention (B=1, S=4096, D=2048, H=16, rotary_dim=64) on 8 TRN2 NeuronCores.

Head-sharded tensor parallelism: each core computes 2 heads end-to-end
(QKV projection + RoPE + full softmax attention) plus its slice of the
row-sharded output projection; the 8 partial [S, D] outputs are summed on
the host.

Precision/perf scheme:
  - All matmuls in fp16 (PE fp16 = full 1 elem/cell/cycle rate; fp8 was
    measured to push its ~3% element error through the softmax average at
    full relative strength -> 6% output error, over the 2e-2 gate).
  - Softmax skips max-subtraction (scores ~N(0,1)): exp via ScalarE in
    2048-wide activations over 4-bank PSUM score quads (amortizes the
    ~293ns per-ACTIVATE overhead). Denominator = serial chain of 7 DVE
    quad-adds + 4 ones-matmuls per (head, q-tile), replacing the old
    DVE/GpSimd pair-add scheme (frees ~220us GpSimd + ~90us DVE).
  - Out-proj partials evicted PSUM->SBUF alternating DVE/ScalarE, written
    to DRAM as fp16; host sums the 8 partials in f32.
"""

import numpy as np

import concourse.bass as bass
import concourse.mybir as mybir
import concourse.tile as tile
from concourse import bacc
from concourse.masks import make_identity
from concourse.bass_utils import run_bass_kernel_spmd
F32 = mybir.dt.float32
FP16 = mybir.dt.float16
FP8 = mybir.dt.float8e4

D = 2048
H = 16
HD = 128
ROT = 64
NCORES = 8
HPC = H // NCORES  # heads per core
SCALE = float(HD) ** -0.5
WS = 1.0  # no operand pre-scale in the all-fp16 scheme

DR = mybir.MatmulPerfMode.DoubleRow

_CACHE = {}


def build_module(S=4096, ST=512, QTL=512):
    """Build the per-core SPMD bass module. Returns compiled nc.

    3-stage schedule to keep the PE busy across phases:
      A: QKV+RoPE for head 0
      B: attention(head0, qt) interleaved with QKV+RoPE(head1) s-tiles
      C: attention(head1, qt) interleaved with the output projection
    """
    NST = S // ST        # QKV s-tiles
    NKT = D // 128       # contraction tiles for QKV
    NQT = S // QTL       # attention q-tiles
    NKC = S // 128       # attention k-chunks
    NQUAD = NKC // 4     # 4-chunk score quads per q-tile
    ETL = 512            # out-proj e-tile
    NET = D // ETL
    assert NST == NQT

    nc = bacc.Bacc(None, target_bir_lowering=False, debug=True)

    xT_d = nc.dram_tensor("xT", [D, S], FP16, kind="ExternalInput")
    w_d = nc.dram_tensor("wsl", [D, 3 * HPC, 128], FP16, kind="ExternalInput")
    wo_d = nc.dram_tensor("wout", [HPC * HD, D], FP16, kind="ExternalInput")
    b_d = nc.dram_tensor("bsl", [128, 3 * HPC], F32, kind="ExternalInput")
    cs_d = nc.dram_tensor("cs", [ROT, 2, S], FP16, kind="ExternalInput")
    y_d = nc.dram_tensor("y", [S, D], FP16, kind="ExternalOutput")

    xT_r = xT_d[:].rearrange("(t p) s -> p t s", p=128)             # [128,16,S]
    w_r = w_d[:].rearrange("(t p) h m -> p t h m", p=128)            # [128,16,6,128]
    wo_r = wo_d[:].rearrange("(h p) e -> p h e", p=128)              # [128,HPC,D]

    with tile.TileContext(nc) as tc:
        with (
            tc.tile_pool(name="persist", bufs=1) as P,
            tc.tile_pool(name="csp", bufs=1) as csp,
            tc.tile_pool(name="xp", bufs=2) as xp,
            tc.tile_pool(name="vtp", bufs=2) as vtp,
            tc.tile_pool(name="rtp", bufs=2) as rtp,
            tc.tile_pool(name="ptp", bufs=4) as ptp,
            tc.tile_pool(name="accp", bufs=2) as accp,
            tc.tile_pool(name="rcp", bufs=2) as rcp,
            tc.tile_pool(name="yp", bufs=3) as yp,
            # PSUM: psA(2 shared slots: QKV groups / V-transpose / out-proj)
            # + score pairs (2x2 banks) + oacc (1) + dn (1) = 8 banks
            tc.tile_pool(name="psA", bufs=2, space="PSUM") as psA,
            tc.tile_pool(name="pss", bufs=2, space="PSUM") as pssp,
            tc.tile_pool(name="pso", bufs=1, space="PSUM") as psop,
            tc.tile_pool(name="psd", bufs=1, space="PSUM") as psdp,
        ):
            QT = [P.tile([128, S], FP16, tag=f"qt{h}", name=f"qt{h}") for h in range(HPC)]
            KT = [P.tile([128, S], FP16, tag=f"kt{h}", name=f"kt{h}") for h in range(HPC)]
            V = [P.tile([128, NKC, 128], FP16, tag=f"v{h}", name=f"v{h}") for h in range(HPC)]
            AT = [P.tile([128, S], FP16, tag=f"at{h}", name=f"at{h}") for h in range(HPC)]
            w_sb = P.tile([128, NKT, 3 * HPC, 128], FP16, tag="wsb", name="w_sb")
            wo_sb = P.tile([128, HPC, D], FP16, tag="wosb", name="wo_sb")
            b_sb = P.tile([128, 3 * HPC], F32)
            identr = P.tile([128, 128], FP16)
            onesw = P.tile([128, 128], FP16)
            cs_sb = csp.tile([ROT, 2, S], FP16, tag="cst", name="cs_sb")

            # Initial loads in consumption order on the two hardware DMA
            # queues (sync/scalar): the first QKV matmul needs w[j=0] and the
            # first quarter of x-tile (0,0); later weights follow behind.
            nkq = NKT // 2
            def w_load(jj):
                for i in range(2):
                    eng = nc.sync if i % 2 == 0 else nc.scalar
                    eng.dma_start(
                        w_sb[:, i * nkq : (i + 1) * nkq, jj : jj + 1, :],
                        w_r[:, i * nkq : (i + 1) * nkq, jj : jj + 1, :])
            w_load(0)
            xt00 = xp.tile([128, NKT, ST], FP16, tag="xt", name="xt_0_0")
            for i in range(4):
                eng = nc.sync if i % 2 == 0 else nc.scalar
                q = NKT // 4
                eng.dma_start(xt00[:, i * q : (i + 1) * q, :],
                              xT_r[:, i * q : (i + 1) * q, bass.ts(0, ST)])
            w_load(1)
            w_load(2)
            nc.sync.dma_start(b_sb[:], b_d[:])
            nc.scalar.dma_start(cs_sb[:, :, 0 : S // 2], cs_d[:, :, 0 : S // 2])
            nc.sync.dma_start(cs_sb[:, :, S // 2 :], cs_d[:, :, S // 2 :])
            for jj in range(3, 6):
                w_load(jj)
            nc.scalar.dma_start(wo_sb[:], wo_r)
            make_identity(nc, identr)
            nc.vector.memset(onesw[:], WS)

            # Warm the PE clock (HAM) during the initial DMAs: ~3.5us of
            # dummy matmuls so the first real matmuls run at 2.4 GHz.
            wm = psdp.tile([128, 512], F32, tag="dn", name="warm")
            for i in range(40):
                nc.tensor.matmul(wm[:, 0:128], onesw[:], onesw[:],
                                 start=(i == 0), stop=(i == 39))
            wmr = rcp.tile([128, 1], F32, tag="rc", name="warmread")
            nc.vector.tensor_copy(wmr[:], wm[:, 0:1])

            def qkv_stile_thunks(h, st):
                """QKV projection (fp8 DoubleRow) + RoPE + V transpose for one
                head / s-tile, returned as emission thunks for interleaving."""
                sl = bass.ts(st, ST)
                j = 3 * h
                state = {}
                thunks = []

                def t_load():
                    if h == 0 and st == 0:
                        state["xt"] = xt00  # preloaded in the preamble
                        return
                    xt = xp.tile([128, NKT, ST], FP16, tag="xt", name=f"xt_{h}_{st}")
                    nc.sync.dma_start(xt[:, 0 : NKT // 2, :], xT_r[:, 0 : NKT // 2, sl])
                    nc.scalar.dma_start(xt[:, NKT // 2 :, :], xT_r[:, NKT // 2 :, sl])
                    state["xt"] = xt
                thunks.append(t_load)

                def t_group_open(which):
                    state[f"ps{which}"] = psA.tile(
                        [128, ST], F32, tag="a", name=f"ps{which}_{h}_{st}")
                def t_mms(which, k0, k1):
                    ps = state[f"ps{which}"]
                    xt = state["xt"]
                    for k in range(k0, k1):
                        nc.tensor.matmul(
                            ps[:], w_sb[:, k, j + which, :], xt[:, k, :],
                            start=(k == 0), stop=(k == NKT - 1),
                        )
                def t_evict_qk(which):
                    dst = QT[h] if which == 0 else KT[h]
                    nc.scalar.activation(
                        dst[:, sl], state[f"ps{which}"][:],
                        mybir.ActivationFunctionType.Identity,
                        bias=b_sb[:, j + which : j + which + 1],
                    )
                    tmp = rtp.tile([ROT, ST], FP16, tag="rtmp", name=f"rt_{h}_{st}_{which}")
                    nc.vector.tensor_copy(tmp[0 : ROT // 2, :], dst[ROT // 2 : ROT, sl])
                    nc.vector.tensor_copy(tmp[ROT // 2 : ROT, :], dst[0 : ROT // 2, sl])
                    nc.vector.tensor_mul(tmp[:], tmp[:], cs_sb[:, 1, sl])
                    nc.vector.tensor_mul(dst[0:ROT, sl], dst[0:ROT, sl], cs_sb[:, 0, sl])
                    nc.vector.tensor_add(dst[0:ROT, sl], dst[0:ROT, sl], tmp[:])
                def t_evict_v():
                    vt = vtp.tile([128, ST], FP16, tag="vt", name=f"vt_{h}_{st}")
                    nc.scalar.activation(
                        vt[:], state["ps2"][:], mybir.ActivationFunctionType.Identity,
                        bias=b_sb[:, j + 2 : j + 3],
                    )
                    state["vt"] = vt
                def t_vtr():
                    ptr = psA.tile([128, ST // 128, 128], FP16, tag="a",
                                   name=f"ptr_{h}_{st}")
                    for sc in range(ST // 128):
                        nc.tensor.transpose(ptr[:, sc, :],
                                            state["vt"][:, bass.ts(sc, 128)], identr[:])
                    nc.scalar.activation(
                        V[h][:, st * (ST // 128) : (st + 1) * (ST // 128), :], ptr[:],
                        mybir.ActivationFunctionType.Copy)

                for which in range(3):
                    thunks.append(lambda w=which: t_group_open(w))
                    for k0 in range(0, NKT, 4):
                        thunks.append(lambda w=which, a=k0: t_mms(w, a, a + 4))
                    if which < 2:
                        thunks.append(lambda w=which: t_evict_qk(w))
                thunks.append(t_evict_v)
                thunks.append(t_vtr)
                return thunks

            def qkv_stile(h, st):
                for t in qkv_stile_thunks(h, st):
                    t()

            NPAIR = NKC // 2

            def attn_iter(h, qt, fillers=()):
                """One attention iteration: 512 queries x full S keys, in
                2-chunk score pairs (2 PSUM banks each, double-buffered so
                QK of pair i+1 overlaps the exp of pair i).
                `fillers` are extra emission thunks interleaved between pairs."""
                fillers = list(fillers)
                fi = 0
                qsl = bass.ts(qt, QTL)
                oacc = psop.tile([128, QTL], F32, tag="oacc", name=f"oacc_{h}_{qt}")
                acc = accp.tile([128, 2, QTL], FP16, tag="acc", name=f"acc_{h}_{qt}")
                pts = {}
                for pair in range(NPAIR + 1):
                    if pair < NPAIR:
                        # scores for 2 k-chunks into a 2-bank PSUM pair
                        pss = pssp.tile([128, 2, QTL], F32, tag="pss",
                                        name=f"pss_{qt}_{h}_{pair}")
                        for c in range(2):
                            nc.tensor.matmul(
                                pss[:, c, :],
                                KT[h][:, bass.ts(2 * pair + c, 128)], QT[h][:, qsl],
                                start=True, stop=True,
                            )
                        pt = ptp.tile([128, 2, QTL], FP16, tag="pt",
                                      name=f"pt_{qt}_{h}_{pair}")
                        pts[pair] = pt
                        nc.scalar.activation(
                            pt[:], pss[:],
                            mybir.ActivationFunctionType.Exp, scale=SCALE / (WS * WS),
                        )
                    while fi < len(fillers) and fi * (NPAIR + 1) <= (pair + 1) * len(fillers):
                        fillers[fi]()
                        fi += 1
                    qd = pair - 1
                    if 0 <= qd < NPAIR:
                        pt = pts.pop(qd)
                        for c in range(2):
                            kd = 2 * qd + c
                            nc.tensor.matmul(
                                oacc[:], V[h][:, kd, :], pt[:, c, :],
                                start=(kd == 0), stop=(kd == NKC - 1),
                            )
                        # denominator accumulation (serial chain on DVE)
                        if qd == 0:
                            pts["acc_prev"] = pt  # defer: first add needs pair 1
                        elif qd == 1:
                            nc.vector.tensor_add(acc[:], pts.pop("acc_prev")[:], pt[:])
                        else:
                            nc.vector.tensor_add(acc[:], acc[:], pt[:])
                while fi < len(fillers):
                    fillers[fi]()
                    fi += 1

                # tail: denominator matmuls + reciprocal + normalize. Returned
                # as thunks so the caller can weave them into the NEXT
                # iteration's filler stream (hides the serial dn/recip chain
                # behind the next iteration's QK/exp ramp-up).
                def t_dn():
                    dn = psdp.tile([128, QTL], F32, tag="dn", name=f"dn_{h}_{qt}")
                    for c in range(2):
                        nc.tensor.matmul(dn[:], onesw[:], acc[:, c, :],
                                         start=(c == 0), stop=(c == 1))
                    state["dn"] = dn
                def t_recip():
                    rc = rcp.tile([128, QTL], F32, tag="rc", name=f"rc_{h}_{qt}")
                    scr = rcp.tile([128, QTL], F32, tag="rcscr", name=f"rs_{h}_{qt}")
                    nc.vector.reciprocal_approx_accurate(rc[:], state["dn"][:], scr[:])
                    state["rc"] = rc
                def t_mul(c4):
                    msl = bass.ds(qt * QTL + c4 * 128, 128)
                    nc.vector.tensor_mul(AT[h][:, msl], oacc[:, bass.ts(c4, 128)],
                                         state["rc"][:, bass.ts(c4, 128)])
                state = {}
                return [t_dn, t_recip] + [lambda c=c: t_mul(c) for c in range(4)]

            def outproj_thunks(qt):
                thunks = []
                for sc4 in range(QTL // 128):
                    for et in range(NET):
                        def blk(sc4=sc4, et=et):
                            ssl = bass.ds(qt * QTL + sc4 * 128, 128)
                            esl = bass.ts(et, ETL)
                            psy = psA.tile([128, ETL], F32, tag="a",
                                           name=f"psy_{qt}_{sc4}_{et}")
                            for h in range(HPC):
                                nc.tensor.matmul(
                                    psy[:], AT[h][:, ssl], wo_sb[:, h, esl],
                                    start=(h == 0), stop=(h == HPC - 1),
                                )
                            yt = yp.tile([128, ETL], FP16, tag="yt",
                                         name=f"yt_{qt}_{sc4}_{et}")
                            k = sc4 * NET + et
                            if k % 2 == 0:
                                nc.vector.tensor_copy(yt[:], psy[:])
                            else:
                                nc.scalar.activation(
                                    yt[:], psy[:], mybir.ActivationFunctionType.Copy)
                            (nc.sync if k % 2 == 0 else nc.scalar).dma_start(
                                y_d[ssl, esl], yt[:])
                        thunks.append(blk)
                return thunks

            # stage A: QKV head 0
            for st in range(NST):
                qkv_stile(0, st)
            # stage B: attention(head0) || QKV head 1; each iteration's tail
            # (dn/recip/normalize) is woven into the next iteration's fillers
            tail = []
            for qt in range(NQT):
                tail = attn_iter(0, qt, fillers=tail + qkv_stile_thunks(1, qt))
            # stage C: attention(head1) || output projection (lagged one
            # q-tile so fillers never wait on the current iteration's AT)
            for qt in range(NQT):
                fill = tail + (outproj_thunks(qt - 1) if qt > 0 else [])
                tail = attn_iter(1, qt, fillers=fill)
            for t in tail + outproj_thunks(NQT - 1):
                t()

    nc.compile()
    return nc


def _host_prep(x, w_qkv, b_qkv, w_out, S):
    """Build per-core input maps."""
    xT = np.ascontiguousarray(x.reshape(S, D).T).astype(np.float16)

    # RoPE tables (match reference._rope_cos_sin)
    inv_freq = (1.0 / (10000.0 ** (np.arange(0, ROT, 2, dtype=np.float32) / ROT))).astype(np.float32)
    t = np.arange(S, dtype=np.float32)
    freqs = np.outer(t, inv_freq)                      # [S, ROT/2]
    emb = np.concatenate([freqs, freqs], axis=-1)      # [S, ROT]
    cosT = np.cos(emb).astype(np.float32).T            # [ROT, S]
    sinT = np.sin(emb).astype(np.float32).T
    sinS = sinT.copy()
    sinS[0 : ROT // 2] *= -1.0
    cs = np.ascontiguousarray(np.stack([cosT, sinS], axis=1)).astype(np.float16)  # [ROT, 2, S]

    in_maps = []
    for c in range(NCORES):
        cols = []
        bcols = []
        for h in [HPC * c + i for i in range(HPC)]:
            for part in range(3):  # q, k, v
                off = part * D + h * HD
                cols.append(w_qkv[:, off : off + HD] * WS)
                bcols.append(b_qkv[off : off + HD] * WS)
        wsl = np.ascontiguousarray(np.stack(cols, axis=1)).astype(np.float16)   # [D, 3*HPC, 128]
        bsl = np.ascontiguousarray(np.stack(bcols, axis=1)).astype(np.float32)   # [128, 3*HPC]
        wout_sl = np.ascontiguousarray(w_out[c * HPC * HD : (c + 1) * HPC * HD, :]).astype(np.float16)
        in_maps.append({"xT": xT, "wsl": wsl, "bsl": bsl, "wout": wout_sl, "cs": cs})
    return in_maps


def kernel(x, w_qkv, b_qkv, w_out, b_out):
    B, S, D_ = x.shape
    assert B == 1 and D_ == D
    if "nc" not in _CACHE:
        _CACHE["nc"] = build_module(S=S)
    nc = _CACHE["nc"]
    in_maps = _host_prep(np.asarray(x, dtype=np.float32), np.asarray(w_qkv, dtype=np.float32),
                         np.asarray(b_qkv, dtype=np.float32), np.asarray(w_out, dtype=np.float32), S)
    res = run_bass_kernel_spmd(nc, in_maps, list(range(NCORES)))
    y = np.zeros((S, D), dtype=np.float32)
    for c in range(NCORES):
        y += res.results[c]["y"].astype(np.float32)
    y += np.asarray(b_out, dtype=np.float32)[None, :]
    return y.reshape(1, S, D)
